# revision 6
# baseline (speedup 1.0000x reference)
"""Trainium2 Bass kernel for CoocOpModel.

out[b,s,z] = sum_{i,j} func[b,s,i] * cooc[i,j,z] * arg[b,s,j]
  with func = func_and_arg[..., :128], arg = func_and_arg[..., 128:]

Shapes (hardcoded): func_and_arg [4,1024,256] f32, cooccurrences [128,128,128] f32,
out [4,1024,128] f32.  D = 128, tokens T = 4096.

Strategy: data-parallel over tokens across 8 cores (512 tokens/core).

Per-core math as ONE flattened contraction over (i,j):
  out[z, t] = sum_{(i,j)} C2[(i,j), z] * P[(i,j), t],  P[(i,j), t] = f[i,t]*a[j,t]

The 16384-long (i,j) axis is processed as 128 PSUM-accumulated matmul
chunks of 128 partition-pairs each.  A chunk covers GI=8 i's x GJ=16 j's
(partition p = ii*16 + jj).  This mixed layout is what makes the moving
operand cheap to build:

  - f slab per I-group:  f_sb[p, t] = f[I*8 + p//16, t]   (each f row
    replicated over only 16 partitions -> 16 slabs x 128KB = 2MB DMA,
    vs 16MB for a full 128-way broadcast)
  - a slabs (2 tiles):   a_all[p, J*512+t] = a[J*16 + p%16, t]  (1MB)
  - P (TT on DVE/Pool):  P[p, (J4,t)] = f_sb[p, t] * a_half[p, (J4,t)]
    with f re-read 4x through a free-dim step-0 AP.  A few TT units run
    on GpSimd to relieve the DVE bottleneck.

Replication slabs are DMA'd straight from DRAM with step-0 dims
(DRAM-source APs allow partition/step-0 replication; SBUF sources don't).

PE: 128 matmuls, stationary = c2r chunk [p=128,(z)128], moving = P
[p=128, t=512], all accumulating into one PSUM bank [128z, 512t] f32.

Host pre-reorder: c2r[ii*16+jj, (I*8+J)*128 + z] = cooc[I*8+ii, J*16+jj, z].
"""

import sys

sys.path.insert(0, "/opt/trn_rl_repo")

import numpy as np
from contextlib import ExitStack

import concourse.bass as bass
import concourse.tile as tile
from concourse import bacc, mybir
from concourse.bass_utils import run_bass_kernel_spmd

F16 = mybir.dt.float16
F32 = mybir.dt.float32
NP_F16 = np.float16

N_CORES = 8
D = 128
T_TOTAL = 4096
T_CORE = T_TOTAL // N_CORES  # 512
GI, GJ = 8, 16               # i's / j's per chunk
NI, NJ = D // GI, D // GJ    # 16 I-groups, 8 J-groups

# TT units (I, half) handled by GpSimd instead of DVE.  Pool's software
# multiply runs ~3.7x slower than DVE per element, so ~1 in 5 units
# balances the two queues.
POOL_UNITS = {2, 7, 12, 17, 22, 27}

_NC_CACHE = None


def _build():
    nc = bacc.Bacc("TRN2", target_bir_lowering=False, debug=False, num_devices=N_CORES)

    f_t = nc.dram_tensor("f_t", [D, T_CORE], F16, kind="ExternalInput").ap()
    a_t = nc.dram_tensor("a_t", [D, T_CORE], F16, kind="ExternalInput").ap()
    # c2r[ii*16+jj, (I*8+J)*128 + z] = cooc[I*8+ii, J*16+jj, z]
    c2 = nc.dram_tensor("c2", [D, D * D], F16, kind="ExternalInput").ap()
    out_t = nc.dram_tensor("out_t", [D, T_CORE], F32, kind="ExternalOutput").ap()

    HALF = 4 * T_CORE  # 2048: four J-chunks per TT unit

    with tile.TileContext(nc) as tc:
        with ExitStack() as ctx:
            const_pool = ctx.enter_context(tc.tile_pool(name="const", bufs=1))
            f_pool = ctx.enter_context(tc.tile_pool(name="fsl", bufs=4))
            c_pool = ctx.enter_context(tc.tile_pool(name="csl", bufs=4))
            p_pool = ctx.enter_context(tc.tile_pool(name="p", bufs=4))
            out_pool = ctx.enter_context(tc.tile_pool(name="out", bufs=1))
            psum_pool = ctx.enter_context(
                tc.tile_pool(name="psum", bufs=1, space="PSUM")
            )

            # a replication slabs: a_half[h][p, J4*512+t] = a[(4h+J4)*16 + p%16, t]
            # (DMA APs are limited to 3 dims -> one DMA per J-chunk)
            a_half0 = const_pool.tile([D, HALF], F16, tag="a0")
            a_half1 = const_pool.tile([D, HALF], F16, tag="a1")
            a_halves = [a_half0, a_half1]

            def a_dma(h, J4):
                a_src = bass.AP(
                    a_t.tensor,
                    (h * 4 + J4) * GJ * T_CORE,
                    [[0, 8], [T_CORE, GJ], [1, T_CORE]],
                )
                nc.sync.dma_start(
                    a_halves[h][:, J4 * T_CORE : (J4 + 1) * T_CORE], a_src
                )

            c_slabs = {}

            def c_dma(I, eng):
                c_sb = c_pool.tile([D, NJ * D], F16, tag=f"c{I}")
                eng.dma_start(c_sb[:], c2[:, I * NJ * D : (I + 1) * NJ * D])
                c_slabs[I] = c_sb

            # head: first a-half, then first c slab, then the rest
            for J4 in range(4):
                a_dma(0, J4)
            c_dma(0, nc.sync)
            for J4 in range(4):
                a_dma(1, J4)

            ps = psum_pool.tile([D, T_CORE], F32)

            q = 0
            for I in range(NI):
                # f slab: f_sb[p, t] = f[I*8 + p//16, t]
                f_sb = f_pool.tile([D, T_CORE], F16, tag="f")
                f_src = bass.AP(
                    f_t.tensor,
                    I * GI * T_CORE,
                    [[T_CORE, GI], [0, GJ], [1, T_CORE]],
                )
                nc.scalar.dma_start(f_sb[:], f_src)

                # prefetch next cooc slab, alternating queues
                if I + 1 < NI:
                    c_dma(I + 1, nc.sync if I % 2 == 0 else nc.scalar)
                c_sb = c_slabs.pop(I)

                f_ap = f_sb[:]
                f_view = bass.AP(
                    f_ap.tensor, f_ap.offset, [f_ap.ap[0], [0, 4], [1, T_CORE]]
                )
                for h in range(2):
                    unit = I * 2 + h
                    pt = p_pool.tile([D, HALF], F16, tag="p")
                    eng = nc.gpsimd if unit in POOL_UNITS else nc.vector
                    eng.tensor_mul(pt[:], f_view, a_halves[h][:])
                    for J4 in range(4):
                        nc.tensor.matmul(
                            ps[:],
                            c_sb[:, (h * 4 + J4) * D : (h * 4 + J4 + 1) * D],
                            pt[:, J4 * T_CORE : (J4 + 1) * T_CORE],
                            start=(q == 0),
                            stop=(q == NI * NJ - 1),
                        )
                        q += 1

            o_sb = out_pool.tile([D, T_CORE], F32, tag="o")
            nc.vector.tensor_copy(o_sb[:], ps[:])
            nc.sync.dma_start(out_t[:, :], o_sb[:])

    nc.compile()
    return nc


def _get_nc():
    global _NC_CACHE
    if _NC_CACHE is None:
        _NC_CACHE = _build()
    return _NC_CACHE


def _prep_in_maps(func_and_arg, cooccurrences):
    fa = np.asarray(func_and_arg, dtype=np.float32).reshape(T_TOTAL, 2 * D)
    c2r = (
        np.asarray(cooccurrences, dtype=np.float32)
        .reshape(NI, GI, NJ, GJ, D)
        .transpose(1, 3, 0, 2, 4)
        .reshape(D, D * D)
        .astype(NP_F16)
    )
    c2r = np.ascontiguousarray(c2r)
    in_maps = []
    for c in range(N_CORES):
        s = fa[c * T_CORE : (c + 1) * T_CORE]  # [512, 256]
        f_tc = np.ascontiguousarray(s[:, :D].T).astype(NP_F16)  # [128 i, 512 t]
        a_tc = np.ascontiguousarray(s[:, D:].T).astype(NP_F16)  # [128 j, 512 t]
        in_maps.append({"f_t": f_tc, "a_t": a_tc, "c2": c2r})
    return in_maps


def kernel(func_and_arg: np.ndarray, cooccurrences: np.ndarray) -> np.ndarray:
    assert func_and_arg.shape == (4, 1024, 2 * D)
    assert cooccurrences.shape == (D, D, D)

    in_maps = _prep_in_maps(func_and_arg, cooccurrences)
    nc = _get_nc()
    res = run_bass_kernel_spmd(nc, in_maps, core_ids=list(range(N_CORES)))

    # out_t per core: [z=128, t=512] -> [t, z]; concat over cores -> [4096, 128]
    outs = [res.results[c]["out_t"].T for c in range(N_CORES)]
    out = np.concatenate(outs, axis=0).reshape(4, 1024, D).astype(np.float32)
    return out


# revision 7
# speedup vs baseline: 1.2149x; 1.2149x over previous
"""Trainium2 Bass kernel for CoocOpModel.

out[b,s,z] = sum_{i,j} func[b,s,i] * cooc[i,j,z] * arg[b,s,j]
  with func = func_and_arg[..., :128], arg = func_and_arg[..., 128:]

Shapes (hardcoded): func_and_arg [4,1024,256] f32, cooccurrences [128,128,128] f32,
out [4,1024,128] f32.  D = 128, tokens T = 4096.

Strategy: data-parallel over tokens across 8 cores (512 tokens/core).

Per-core math as ONE flattened contraction over (i,j):
  out[z, t] = sum_{(i,j)} C2[(i,j), z] * P[(i,j), t],  P[(i,j), t] = f[i,t]*a[j,t]

The 16384-long (i,j) axis is processed as 128 PSUM-accumulated matmul
chunks of 128 partition-pairs each.  A chunk covers GI=8 i's x GJ=16 j's
(partition p = ii*16 + jj).  This mixed layout is what makes the moving
operand cheap to build:

  - f slab per I-group:  f_sb[p, t] = f[I*8 + p//16, t]   (each f row
    replicated over only 16 partitions -> 16 slabs x 128KB = 2MB DMA,
    vs 16MB for a full 128-way broadcast)
  - a slabs (2 tiles):   a_all[p, J*512+t] = a[J*16 + p%16, t]  (1MB)
  - P (TT on DVE):       P[p, (J4,t)] = f_sb[p, t] * a_half[p, (J4,t)]
    with f re-read 4x through a free-dim step-0 AP.

The first TT unit is a single chunk fed by a small dedicated a_j0 tile so
the PE chain starts as soon as ~380KB (not ~900KB) of DMA has landed.

Replication slabs are DMA'd straight from DRAM with step-0 dims
(DRAM-source APs allow partition/step-0 replication; SBUF sources don't).

PE: 128 matmuls, stationary = c2r chunk [p=128,(z)128], moving = P
[p=128, t=512], all accumulating into one PSUM bank [128z, 512t] f32.

Host pre-reorder: c2r[ii*16+jj, (I*8+J)*128 + z] = cooc[I*8+ii, J*16+jj, z].
"""

import sys

sys.path.insert(0, "/opt/trn_rl_repo")

import numpy as np
from contextlib import ExitStack

import concourse.bass as bass
import concourse.tile as tile
from concourse import bacc, mybir
from concourse.bass_utils import run_bass_kernel_spmd

F16 = mybir.dt.float16
F32 = mybir.dt.float32
NP_F16 = np.float16

N_CORES = 8
D = 128
T_TOTAL = 4096
T_CORE = T_TOTAL // N_CORES  # 512
GI, GJ = 8, 16               # i's / j's per chunk
NI, NJ = D // GI, D // GJ    # 16 I-groups, 8 J-groups

_NC_CACHE = None


def _build():
    nc = bacc.Bacc("TRN2", target_bir_lowering=False, debug=False, num_devices=N_CORES)

    f_t = nc.dram_tensor("f_t", [D, T_CORE], F16, kind="ExternalInput").ap()
    a_t = nc.dram_tensor("a_t", [D, T_CORE], F16, kind="ExternalInput").ap()
    # c2r[ii*16+jj, (I*8+J)*128 + z] = cooc[I*8+ii, J*16+jj, z]
    c2 = nc.dram_tensor("c2", [D, D * D], F16, kind="ExternalInput").ap()
    out_t = nc.dram_tensor("out_t", [D, T_CORE], F32, kind="ExternalOutput").ap()

    HALF = 4 * T_CORE  # 2048: four J-chunks per TT unit

    with tile.TileContext(nc) as tc:
        with ExitStack() as ctx:
            const_pool = ctx.enter_context(tc.tile_pool(name="const", bufs=1))
            f_pool = ctx.enter_context(tc.tile_pool(name="fsl", bufs=4))
            c_pool = ctx.enter_context(tc.tile_pool(name="csl", bufs=4))
            p_pool = ctx.enter_context(tc.tile_pool(name="p", bufs=6))
            out_pool = ctx.enter_context(tc.tile_pool(name="out", bufs=1))
            psum_pool = ctx.enter_context(
                tc.tile_pool(name="psum", bufs=1, space="PSUM")
            )

            def a_src(J):
                return bass.AP(
                    a_t.tensor,
                    J * GJ * T_CORE,
                    [[0, 8], [T_CORE, GJ], [1, T_CORE]],
                )

            # small dedicated first-chunk a tile: a_j0[p, t] = a[p%16, t]
            a_j0 = const_pool.tile([D, T_CORE], F16, tag="aj0")
            nc.sync.dma_start(a_j0[:], a_src(0))

            # a replication slabs: a_half[h][p, J4*512+t] = a[(4h+J4)*16 + p%16, t]
            # (DMA APs are limited to 3 dims -> one DMA per J-chunk)
            a_half0 = const_pool.tile([D, HALF], F16, tag="a0")
            a_half1 = const_pool.tile([D, HALF], F16, tag="a1")
            a_halves = [a_half0, a_half1]
            for h in range(2):
                for J4 in range(4):
                    nc.sync.dma_start(
                        a_halves[h][:, J4 * T_CORE : (J4 + 1) * T_CORE],
                        a_src(h * 4 + J4),
                    )

            c_slabs = {}

            def c_dma(I, eng):
                c_sb = c_pool.tile([D, NJ * D], F16, tag=f"c{I}")
                eng.dma_start(c_sb[:], c2[:, I * NJ * D : (I + 1) * NJ * D])
                c_slabs[I] = c_sb

            # f slab: f_sb[p, t] = f[I*8 + p//16, t]
            f_slabs = {}

            def f_dma(I):
                f_sb = f_pool.tile([D, T_CORE], F16, tag="f")
                f_src = bass.AP(
                    f_t.tensor,
                    I * GI * T_CORE,
                    [[T_CORE, GI], [0, GJ], [1, T_CORE]],
                )
                nc.scalar.dma_start(f_sb[:], f_src)
                f_slabs[I] = f_sb

            # head: first f slab and first cooc slab on the scalar queue so
            # they race the a tiles on the sync queue
            f_dma(0)
            c_dma(0, nc.scalar)

            ps = psum_pool.tile([D, T_CORE], F32)

            q = 0
            for I in range(NI):
                if I not in f_slabs:
                    f_dma(I)
                f_sb = f_slabs.pop(I)
                # prefetch next cooc slab, alternating queues
                if I + 1 < NI:
                    c_dma(I + 1, nc.sync if I % 2 == 1 else nc.scalar)
                c_sb = c_slabs.pop(I)

                f_ap = f_sb[:]

                def f_view(reps):
                    return bass.AP(
                        f_ap.tensor, f_ap.offset, [f_ap.ap[0], [0, reps], [1, T_CORE]]
                    )

                if I == 0:
                    # split the first unit: chunk (0,0) alone off the small
                    # a_j0 tile, then chunks (0,1..3) off a_half0
                    p0 = const_pool.tile([D, T_CORE], F16, tag="p0")
                    nc.vector.tensor_mul(p0[:], f_ap, a_j0[:])
                    nc.tensor.matmul(
                        ps[:], c_sb[:, 0:D], p0[:], start=True, stop=False
                    )
                    q += 1
                    p1 = const_pool.tile([D, 3 * T_CORE], F16, tag="p1")
                    nc.vector.tensor_mul(
                        p1[:], f_view(3), a_half0[:, T_CORE : 4 * T_CORE]
                    )
                    for J4 in range(1, 4):
                        nc.tensor.matmul(
                            ps[:],
                            c_sb[:, J4 * D : (J4 + 1) * D],
                            p1[:, (J4 - 1) * T_CORE : J4 * T_CORE],
                            start=False,
                            stop=False,
                        )
                        q += 1
                    halves = [1]
                else:
                    halves = [0, 1]

                for h in halves:
                    pt = p_pool.tile([D, HALF], F16, tag="p")
                    nc.vector.tensor_mul(pt[:], f_view(4), a_halves[h][:])
                    for J4 in range(4):
                        nc.tensor.matmul(
                            ps[:],
                            c_sb[:, (h * 4 + J4) * D : (h * 4 + J4 + 1) * D],
                            pt[:, J4 * T_CORE : (J4 + 1) * T_CORE],
                            start=False,
                            stop=(q == NI * NJ - 1),
                        )
                        q += 1

            o_sb = out_pool.tile([D, T_CORE], F32, tag="o")
            nc.vector.tensor_copy(o_sb[:], ps[:])
            nc.sync.dma_start(out_t[:, :], o_sb[:])

    nc.compile()
    return nc


def _get_nc():
    global _NC_CACHE
    if _NC_CACHE is None:
        _NC_CACHE = _build()
    return _NC_CACHE


def _prep_in_maps(func_and_arg, cooccurrences):
    fa = np.asarray(func_and_arg, dtype=np.float32).reshape(T_TOTAL, 2 * D)
    c2r = (
        np.asarray(cooccurrences, dtype=np.float32)
        .reshape(NI, GI, NJ, GJ, D)
        .transpose(1, 3, 0, 2, 4)
        .reshape(D, D * D)
        .astype(NP_F16)
    )
    c2r = np.ascontiguousarray(c2r)
    in_maps = []
    for c in range(N_CORES):
        s = fa[c * T_CORE : (c + 1) * T_CORE]  # [512, 256]
        f_tc = np.ascontiguousarray(s[:, :D].T).astype(NP_F16)  # [128 i, 512 t]
        a_tc = np.ascontiguousarray(s[:, D:].T).astype(NP_F16)  # [128 j, 512 t]
        in_maps.append({"f_t": f_tc, "a_t": a_tc, "c2": c2r})
    return in_maps


def kernel(func_and_arg: np.ndarray, cooccurrences: np.ndarray) -> np.ndarray:
    assert func_and_arg.shape == (4, 1024, 2 * D)
    assert cooccurrences.shape == (D, D, D)

    in_maps = _prep_in_maps(func_and_arg, cooccurrences)
    nc = _get_nc()
    res = run_bass_kernel_spmd(nc, in_maps, core_ids=list(range(N_CORES)))

    # out_t per core: [z=128, t=512] -> [t, z]; concat over cores -> [4096, 128]
    outs = [res.results[c]["out_t"].T for c in range(N_CORES)]
    out = np.concatenate(outs, axis=0).reshape(4, 1024, D).astype(np.float32)
    return out
